# revision 39
# baseline (speedup 1.0000x reference)
"""Block-sparse linear layer (x @ (mask*W).T + bias) on 8 TRN2 NeuronCores.

Strategy: data-parallel over batch rows. Each core gets 1024 rows of x
(transposed to [k, m] on host, cast to bf16), the packed kept weight
blocks (bf16), and bias. On-device: out.T tile [o=128, m=1024] accumulates
in PSUM over the 16 kept k-subtiles (k-subtile = 128 rows), with W tiles
stationary and x slabs moving. PSUM is evicted through the vector/scalar
engines with the per-partition bias add fused, then DMA'd out. The host
reassembles the full [8192, 4096] fp32 output.

Startup is DMA-latency critical: all startup transfers ride the Sync
queue's DMA ring (the only fast one, ~105 GB/s per DMA / ~310 aggregate;
the GpSimd/Scalar rings measured 25-40 GB/s), finely split and issued in
strict PE-consumption order, while the bulk weight stream (t>=7, GpSimd
queue) is throttled by a 7-deep weight pool so its buffer-reuse
semaphores keep it out of the startup window. Ten gapless dummy matmuls
fire the HAM clock un-throttle (a fully-busy ~3.4us window -> 2.4 GHz)
before the DMA-paced real stream begins, with margin so supply jitter
cannot open a re-throttling idle window. The last two o-tiles are
computed and evicted in column pieces on separate single-bank PSUM tiles
(vector/scalar engines, pieces' out-DMAs on the fast ring) so only a
quarter-tile eviction and one small DMA trail the final matmul.
"""

import sys
import types

import numpy as np
import ml_dtypes

BATCH = 8192
SIZE = 4096
NB = 16
BLOCK = 256
NCORES = 8
MC = BATCH // NCORES  # 1024 rows per core
P = 128
KS = SIZE // P  # 32 k-subtiles
OT = SIZE // P  # 32 o-tiles
MM_N = 512  # moving free dim per matmul
XG = 2  # x chunks per DMA group (steady state)

_BUILD_CACHE = {}


def _install_ntff_hook():
    # Register the axon NTFF profiling hook if the image's antenv lacks it.
    if "antenv.axon_hooks" in sys.modules:
        return
    try:
        from trn_agent_boot.trn_boot import _ntff_profile_via_ctypes

        hook = _ntff_profile_via_ctypes("/opt/axon/libaxon_pjrt.so")
        mod = types.ModuleType("antenv.axon_hooks")
        mod.get_axon_ntff_profile_hook = lambda: hook
        sys.modules["antenv.axon_hooks"] = mod
    except Exception:
        pass


def _block_keep_from_mask(mask):
    """Return [NB, NB] bool of kept blocks if mask is block-constant, else None."""
    m4 = np.asarray(mask).reshape(NB, BLOCK, NB, BLOCK)
    keep = m4[:, 0, :, 0]
    uniform = np.all(m4 == keep[:, None, :, None])
    return keep if uniform else None


def _ks_lists(keep):
    """Per o-tile (128 outputs) list of kept k-subtile indices, padded to
    a uniform length (padding points at subtile 0 with zero weights).
    Each tile's list is sorted by global first-use (DMA/consumption)
    order, so the startup weight-piece DMAs (packed s-ranges) cover
    exactly the chunks the PE needs next."""
    lists = []
    for t in range(OT):
        i = (t * P) // BLOCK  # o-block row
        ks = []
        for j in range(NB):
            if keep[i, j]:
                base = (j * BLOCK) // P
                ks.extend(range(base, base + BLOCK // P))
        lists.append(ks)
    ks_order = []
    for l in lists:
        for ks in l:
            if ks not in ks_order:
                ks_order.append(ks)
    lists = [sorted(l, key=ks_order.index) for l in lists]
    n_sub = max(1, max(len(l) for l in lists))
    padded = tuple(tuple(l + [-1] * (n_sub - len(l))) for l in lists)
    return padded, n_sub


def _fp8_pairs(ks_lists):
    """Per-tile pair of k-subtiles computed via one fp8 DoubleRow matmul
    (the two consumed last). Tiles 0-3 (startup-critical first block) and
    the last two (pure-bf16 tail block) are excluded."""
    pair_of = {}
    pair_ids = {}
    for t in range(4, OT - 2):
        valid = [c for c in ks_lists[t] if c >= 0]
        pairs = []
        if len(valid) >= 6:
            pairs.append((valid[-4], valid[-3]))
        if len(valid) >= 4:
            pairs.append((valid[-2], valid[-1]))
        if not pairs:
            continue
        pair_of[t] = pairs
        for pair in pairs:
            if pair not in pair_ids:
                pair_ids[pair] = len(pair_ids)
    return pair_of, pair_ids


def _build(ks_lists, n_sub):
    import concourse.mybir as mybir
    import concourse.tile as tile
    from concourse import bacc

    bf16, f32 = mybir.dt.bfloat16, mybir.dt.float32
    fp8 = mybir.dt.float8e4
    pair_of, pair_ids = _fp8_pairs(ks_lists)
    np8 = max(1, len(pair_ids))
    nc = bacc.Bacc("TRN2", target_bir_lowering=False)
    xt_d = nc.declare_dram_parameter("xt", [P, KS, MC], bf16, isOutput=False)
    wt_d = nc.declare_dram_parameter("wt", [OT, P, n_sub, P], bf16, isOutput=False)
    x8_d = nc.declare_dram_parameter("x8", [np8, P, 2, MC], fp8, isOutput=False)
    w8_d = nc.declare_dram_parameter("w8", [P, OT * 2, 2, P], fp8, isOutput=False)
    bias_d = nc.declare_dram_parameter("biast", [P, OT], f32, isOutput=False)
    out_d = nc.declare_dram_parameter("out", [OT, P, MC], f32, isOutput=True)

    # x DMA issue order: k-subtiles in order of first use across o-tiles.
    ks_order = []
    for t in range(OT):
        for ks in ks_lists[t]:
            if ks >= 0 and ks not in ks_order:
                ks_order.append(ks)
    for ks in range(KS):
        if ks not in ks_order:
            ks_order.append(ks)

    # 7 weight buffers: w0..w6 load freely; w7+ (GpSimd stream) is gated by
    # buffer-reuse semaphores, keeping it out of the startup DMA window.
    W_POOL_BUFS = 7
    HALF = MC // 2

    with tile.TileContext(nc) as tc:
        with (
            tc.tile_pool(name="const", bufs=1) as const_pool,
            tc.tile_pool(name="xpool", bufs=1) as xpool,
            tc.tile_pool(name="wpool", bufs=W_POOL_BUFS) as wpool,
            tc.tile_pool(name="opool", bufs=3) as opool,
            tc.tile_pool(name="psum", bufs=4, space="PSUM") as psum_pool,
        ):
            bias_tile = const_pool.tile([P, OT], f32)
            nc.gpsimd.dma_start(out=bias_tile[:], in_=bias_d[:])

            # Warm the PE clock with GAPLESS dummy matmuls while the first
            # x/W DMAs are in flight. HAM un-throttles (1.2->2.4 GHz) only
            # after a fully-busy ~3.4us window; the DMA-paced early real
            # matmuls have small gaps that keep resetting it, so fire the
            # full window here (8 x N=512 back-to-back ~= 3.6us). Once
            # K=8/8, only a >=3.4us idle re-throttles — the sub-microsecond
            # supply gaps later never do.
            warm = const_pool.tile([P, MM_N], bf16, name="warm")
            nc.vector.memset(warm[:], 0)
            warm_ps = psum_pool.tile([P, MM_N], f32, name="warm_ps", tag="ps")
            N_WARM = 10
            for i in range(N_WARM):
                nc.tensor.matmul(
                    warm_ps[:],
                    lhsT=warm[:, 0:P],
                    rhs=warm[:],
                    start=(i == 0),
                    stop=(i == N_WARM - 1),
                )

            w_tiles = {}

            def w_alloc(t):
                w_tiles[t] = wpool.tile([P, n_sub, P], bf16, name="w_tile")

            def w_dma(t, engine, half=None, srange=None):
                if t not in w_tiles:
                    w_alloc(t)
                w = w_tiles[t]
                if srange is not None:
                    lo, hi = srange
                elif half is None:
                    lo, hi = 0, n_sub
                else:
                    step = (n_sub + 1) // 2
                    lo, hi = half * step, min((half + 1) * step, n_sub)
                engine.dma_start(
                    out=w[:, lo:hi, :], in_=wt_d[t, :, lo:hi, :]
                )

            x_ap = {}
            x_groups = []
            N_SINGLE = 8  # first chunks land individually for smooth supply
            pos = 0
            while pos < len(ks_order):
                n = 1 if len(x_groups) < N_SINGLE else XG
                n = min(n, len(ks_order) - pos)
                grp = ks_order[pos : pos + n]
                lo = min(grp)
                assert grp == list(range(lo, lo + len(grp))), grp
                x_groups.append((lo, len(grp)))
                pos += n

            def x_dma(gi):
                lo, n = x_groups[gi]
                xg = xpool.tile([P, n, MC], bf16, name=f"x_g{gi}", uniquify=False)
                nc.sync.dma_start(out=xg[:], in_=xt_d[:, lo : lo + n, :])
                for off in range(n):
                    x_ap[lo + off] = xg[:, off, :]

            # Startup-critical DMAs all on the Sync queue (the only fast
            # ring: ~105 GB/s per DMA, ~310 GB/s aggregate, prompt start;
            # GpSimd/Scalar software rings measured 25-35 GB/s). The ring
            # fair-shares bandwidth over all in-flight transfers, so issue
            # in STRICT PE-consumption order, finely split at the front:
            # x chunk 16 cols 0-511 + w0 subtiles 0-3 gate matmul #1.
            c0_lo, _ = x_groups[0]
            xg0 = xpool.tile([P, 1, MC], bf16, name="x_g0", uniquify=False)
            nc.sync.dma_start(
                out=xg0[:, 0:1, 0:HALF], in_=xt_d[:, c0_lo : c0_lo + 1, 0:HALF]
            )
            x_ap[c0_lo] = xg0[:, 0, :]
            h0n = (n_sub + 1) // 2
            q0n = (h0n + 1) // 2
            w_dma(0, nc.sync, srange=(0, q0n))
            nc.sync.dma_start(
                out=xg0[:, 0:1, HALF:MC], in_=xt_d[:, c0_lo : c0_lo + 1, HALF:MC]
            )
            w_dma(1, nc.sync, srange=(0, q0n))
            w_dma(2, nc.sync, srange=(0, q0n))
            w_dma(3, nc.sync, srange=(0, q0n))
            x_dma(1)
            w_dma(2, nc.sync, srange=(q0n, h0n))
            w_dma(3, nc.sync, srange=(q0n, h0n))
            x_dma(2)
            w_dma(0, nc.sync, srange=(q0n, h0n))
            w_dma(1, nc.sync, srange=(q0n, h0n))
            x_dma(3)
            x_dma(4)
            w_dma(2, nc.sync, half=1)
            w_dma(3, nc.sync, half=1)
            x_dma(5)
            w_dma(0, nc.sync, half=1)
            w_dma(1, nc.sync, half=1)
            x_dma(6)
            x_dma(7)
            x_dma(8)
            x_dma(9)
            w_dma(4, nc.sync)
            x_dma(10)
            x_dma(11)
            w_dma(5, nc.sync)
            for gi in range(12, len(x_groups)):
                x_dma(gi)
            w_dma(6, nc.sync)

            # fp8 DoubleRow operands (first needed at ~54us, pair block 2).
            x8_tiles = {}
            for pair, pid in sorted(pair_ids.items(), key=lambda kv: kv[1]):
                x8 = xpool.tile([P, 2, MC], fp8, name=f"x8_{pid}", uniquify=False)
                nc.sync.dma_start(out=x8[:], in_=x8_d[pid, :, :, :])
                x8_tiles[pair] = x8
            w8_all = const_pool.tile([P, OT * 2, 2, P], fp8, name="w8_all")
            nc.sync.dma_start(out=w8_all[:], in_=w8_d[:])

            def emit_block(ts, interleave):
                """Emit the accumulation + eviction for o-tiles `ts`.

                interleave=True: chunk-major across the tiles (each arriving
                x chunk is consumed by every tile that uses it — PE executes
                strictly in order, so this is what absorbs DMA latency).
                interleave=False: tile-major (first tile finishes early so
                its eviction overlaps the next tile's matmuls).
                """
                ps = {t: psum_pool.tile([P, MC], f32, name="ps") for t in ts}
                sets = {t: {ks: s for s, ks in enumerate(ks_lists[t]) if ks >= 0} for t in ts}
                for t in ts:
                    if t in pair_of:  # handled by DoubleRow MMs per half
                        for pair in pair_of[t]:
                            for c in pair:
                                del sets[t][c]
                    if not sets[t]:  # fully-masked o-tile: zero the PSUM
                        sets[t] = {ks_order[0]: 0}
                n_done = {(t, h): 0 for t in ts for h in range(MC // MM_N)}
                if interleave:
                    # h outer within each chunk so consecutive matmuls never
                    # share the stationary operand (each LDWEIGHTS is live).
                    order = [
                        (c, t, h)
                        for c in ks_order
                        for h in range(MC // MM_N)
                        for t in ts
                        if c in sets[t]
                    ]
                else:
                    order = [
                        (c, t, h)
                        for t in ts
                        for c in ks_lists[t]
                        if c >= 0
                        for h in range(MC // MM_N)
                    ]
                for c, t, h in order:
                    s = sets[t][c]
                    first = n_done[(t, h)] == 0
                    n_done[(t, h)] += 1
                    last = n_done[(t, h)] == len(sets[t]) and t not in pair_of
                    nc.tensor.matmul(
                        ps[t][:, h * MM_N : (h + 1) * MM_N],
                        lhsT=w_tiles[t][:, s, :],
                        rhs=x_ap[c][:, h * MM_N : (h + 1) * MM_N],
                        start=first,
                        stop=last,
                    )
                    if not interleave and last and h == MC // MM_N - 1:
                        _evict(ts, t, ps)
                # Close fp8 tiles' accumulations: one DoubleRow matmul per
                # column-half covers the last two k-subtiles (K=256 at 2
                # MACs/cell/cycle), interleaved across tiles so each 256-col
                # weight load hides behind the neighbor's matmul.
                drs = [t for t in ts if t in pair_of]
                for h in range(MC // MM_N):
                    for t in drs:
                        npair = len(pair_of[t])
                        for pi, pair in enumerate(pair_of[t]):
                            nc.tensor.matmul(
                                ps[t][:, h * MM_N : (h + 1) * MM_N],
                                lhsT=w8_all[:, t * 2 + pi, :, :],
                                rhs=x8_tiles[pair][:, :, h * MM_N : (h + 1) * MM_N],
                                start=False,
                                stop=(pi == npair - 1),
                                perf_mode=mybir.MatmulPerfMode.DoubleRow,
                            )
                for t in ts:
                    if t in drs:
                        _evict(ts, t, ps)
                    elif interleave:
                        _evict(ts, t, ps)

            def _evict(ts, t, ps):
                # Evict in halves (out-DMA of the first half overlaps the
                # bias-add of the second); even o-tiles on the Vector
                # engine, odd on Scalar, so neighbor evictions parallelize.
                o_tile = opool.tile([P, MC], f32, name="o_tile")
                half = MC // 2
                for h in range(2):
                    sl = slice(h * half, (h + 1) * half)
                    if t % 2 == 0:
                        nc.vector.tensor_scalar_add(
                            o_tile[:, sl], ps[t][:, sl], bias_tile[:, t : t + 1]
                        )
                    else:
                        nc.scalar.add(
                            o_tile[:, sl], ps[t][:, sl], bias_tile[:, t : t + 1]
                        )
                    nc.sync.dma_start(out=out_d[t, :, sl], in_=o_tile[:, sl])

            def emit_block_hsplit(ts):
                """Tail block: per tile, accumulate column-half 0 into its
                own single-bank PSUM tile, evict it on Vector while half
                1's matmuls stream into a different bank's tile (separate
                tiles, so Tile adds no false eviction->matmul dependency),
                then evict half 1 on Scalar. The very last out-DMA is split
                across two queues to halve its transfer time."""
                sets = {t: {ks: s for s, ks in enumerate(ks_lists[t]) if ks >= 0} for t in ts}
                for t in ts:
                    if not sets[t]:
                        sets[t] = {ks_order[0]: 0}
                for t in ts:
                    o_tile = opool.tile([P, MC], f32, name="o_tile")
                    cs = [c for c in ks_lists[t] if c >= 0] or [ks_order[0]]
                    # Last tile: second half in two N=256 quarters so only
                    # a quarter's eviction + 128KB DMA trails the last MM.
                    if t == ts[-1]:
                        pieces = [(0, HALF), (HALF, HALF + 320), (HALF + 320, MC)]
                    else:
                        pieces = [(0, HALF), (HALF, MC)]
                    for pi, (lo, hi) in enumerate(pieces):
                        sl = slice(lo, hi)
                        # Same shape as the pool's other tiles (a distinct
                        # shape would get its own buffer slots and overflow
                        # PSUM); only the head of the buffer is used.
                        psh = psum_pool.tile([P, MC], f32, name="ps_h", tag="ps")[:, 0 : hi - lo]
                        for i, c in enumerate(cs):
                            nc.tensor.matmul(
                                psh,
                                lhsT=w_tiles[t][:, sets[t][c], :],
                                rhs=x_ap[c][:, sl],
                                start=(i == 0),
                                stop=(i == len(cs) - 1),
                            )
                        if pi == 0:
                            nc.vector.tensor_scalar_add(
                                o_tile[:, sl], psh, bias_tile[:, t : t + 1]
                            )
                            nc.sync.dma_start(out=out_d[t, :, sl], in_=o_tile[:, sl])
                        else:
                            nc.scalar.add(
                                o_tile[:, sl], psh, bias_tile[:, t : t + 1]
                            )
                            # Always the fast Sync ring near the end (the
                            # GpSimd ring is ~3x slower); GpSimd only for
                            # the earlier tile whose transfer has slack.
                            if t == ts[-1]:
                                nc.sync.dma_start(
                                    out=out_d[t, :, sl], in_=o_tile[:, sl]
                                )
                            else:
                                nc.gpsimd.dma_start(
                                    out=out_d[t, :, sl], in_=o_tile[:, sl]
                                )

            # First four o-tiles as one interleaved block (their k-chunk
            # sets overlap heavily, maximizing PE work per arriving byte
            # during the x load); middle o-tiles pair-wise; last pair
            # column-half-split so almost no eviction trails the last MM.
            emit_block((0, 1, 2, 3), interleave=True)
            for pair in range(2, OT // 2):
                ts = (2 * pair, 2 * pair + 1)
                for t in ts:
                    if t >= 7:
                        w_dma(t, nc.gpsimd)
                if pair != OT // 2 - 1:
                    emit_block(ts, interleave=True)
                else:
                    emit_block_hsplit(ts)
    nc.compile()
    return nc


def _get_kernel(ks_lists, n_sub):
    key = (ks_lists, n_sub)
    if key not in _BUILD_CACHE:
        _BUILD_CACHE[key] = _build(ks_lists, n_sub)
    return _BUILD_CACHE[key]


def kernel(x, weight, bias, mask, _trace=False):
    from concourse.bass_utils import run_bass_kernel_spmd

    _install_ntff_hook()

    x = np.asarray(x)
    weight = np.asarray(weight)
    bias = np.asarray(bias, dtype=np.float32)
    keep = _block_keep_from_mask(mask)
    if keep is None:
        # Mask not block-constant: fall back to a dense schedule with the
        # element-masked weights and every k-subtile kept.
        weight = np.where(np.asarray(mask), weight, 0.0).astype(np.float32)
        keep = np.ones((NB, NB), dtype=bool)
    ks_lists, n_sub = _ks_lists(keep)

    nc = _get_kernel(ks_lists, n_sub)

    # Pack weights: wt[t, p, s, q] = W[t*P + q, ks*P + p] for kept subtile ks.
    w4 = weight.reshape(OT, P, KS, P)  # [t, q, ks, p]
    wt = np.zeros((OT, P, n_sub, P), dtype=ml_dtypes.bfloat16)
    for t in range(OT):
        idx = [ks for ks in ks_lists[t]]
        valid = [s for s, ks in enumerate(idx) if ks >= 0]
        sel = w4[t][:, [idx[s] for s in valid], :]  # [q, s_valid, p]
        wt[t][:, valid, :] = sel.transpose(2, 1, 0).astype(ml_dtypes.bfloat16)

    biast = np.ascontiguousarray(
        bias.reshape(OT, P).T, dtype=np.float32
    )  # [P, OT]

    # fp8 operands (ml_dtypes.float8_e4m3 matches TRN FP8_EXP4 bit-for-bit;
    # values here are far below the 240 max-normal).
    pair_of, pair_ids = _fp8_pairs(ks_lists)
    np8 = max(1, len(pair_ids))
    w8 = np.zeros((P, OT * 2, 2, P), dtype=ml_dtypes.float8_e4m3)
    for t, pairs in pair_of.items():
        for pi, pair in enumerate(pairs):
            sel = w4[t][:, list(pair), :]  # [q, 2, p] fp32
            w8[:, t * 2 + pi, :, :] = sel.transpose(2, 1, 0).astype(
                ml_dtypes.float8_e4m3
            )

    in_maps = []
    for c in range(NCORES):
        xc = x[c * MC : (c + 1) * MC, :]  # [MC, SIZE] fp32
        xt_f = np.ascontiguousarray(
            xc.reshape(MC, KS, P).transpose(2, 1, 0)
        )  # [P, KS, MC] fp32
        xt = xt_f.astype(ml_dtypes.bfloat16)
        x8 = np.zeros((np8, P, 2, MC), dtype=ml_dtypes.float8_e4m3)
        for pair, pid in pair_ids.items():
            x8[pid] = xt_f[:, list(pair), :].astype(ml_dtypes.float8_e4m3)
        in_maps.append({"xt": xt, "wt": wt, "x8": x8, "w8": w8, "biast": biast})

    res = run_bass_kernel_spmd(nc, in_maps, list(range(NCORES)), trace=_trace)

    out = np.empty((BATCH, SIZE), dtype=np.float32)
    for c in range(NCORES):
        o = res.results[c]["out"]  # [OT, P, MC]
        out[c * MC : (c + 1) * MC, :] = o.reshape(SIZE, MC).T
    if _trace:
        return out, res
    return out
